# revision 1
# baseline (speedup 1.0000x reference)
"""Type-2 NUFFT (image -> non-uniform k-space) on 8 Trainium2 NeuronCores.

kspace[b,m] = sum_{x,y} image[b,x,y] * exp(-i*(kx_m*(x-128) + ky_m*(y-128)))

Per core (M sharded 8 ways -> 2048 points):
  The image is folded even/odd along y (y'=y-128) into a concatenated rhs
  img_oe = [odd(129) | even(129)] so stage 1 is 2 fp32 matmul chains per
  (batch, m-tile):   S-chain: SxT.T @ img_oe -> [B_odd | B_even]
                     C-chain: CxT.T @ img_oe -> [A_odd | A_even]
  Stage 2 is one fused DVE multiply+row-reduce per output component using
  strided access patterns over the PSUM banks and a shared trig table
  W = [-Sy' | Cy | Sy']:
     Re[m]  = sum(B_odd*-Sy') + sum(A_even*Cy)
     -Im[m] = sum(B_even*Cy)  + sum(A_odd*Sy')

Trig tables on-chip: P = k*grid/(2pi); f = P - round(P) via the fp32
magic-constant trick; sin = Sin(2pi*f) on ScalarE (LUT valid on [-pi,pi]);
cos(2pi*f) = Sin(-2pi*|f| + pi/2) (stays inside the LUT domain).
"""

import sys

if '/opt/trn_rl_repo' not in sys.path:
    sys.path.insert(0, '/opt/trn_rl_repo')

import numpy as np

B, NX, NY, M, NCORES = 2, 256, 256, 16384, 8
ML = M // NCORES            # 2048 m-points per core
NT = ML // 128              # 16 m-tiles per core
TWO_PI = float(2.0 * np.pi)
PI = float(np.pi)
MAGIC = 12582912.0          # 1.5 * 2**23: (x + MAGIC) - MAGIC == round(x) fp32
NS = 129                    # one fold segment (incl pad/singles)
NSEG = 2 * NS               # 258: [odd | even] rhs width
NW = 3 * NS                 # 387: [-Sy' | Cy | Sy']

_CACHE = {}
USE_F32R = True


def _consts():
    xs = (np.arange(NX, dtype=np.float64) - NX // 2) / (2.0 * np.pi)
    xs_cols = xs.astype(np.float32).reshape(2, 128).T.copy()  # [128, 2]
    # y args (in "turns"): odd segment [1..127, -128, 0(pad)], even [0..128]
    ys_odd = np.concatenate([np.arange(1, 128), [-128.0], [0.0]])
    ys_even = np.arange(0, 129, dtype=np.float64)
    yargs = (np.concatenate([ys_odd, ys_even]) / (2.0 * np.pi)).astype(np.float32)
    ysb = np.broadcast_to(yargs.reshape(1, NSEG), (128, NSEG)).copy()
    ident16 = np.eye(16, dtype=np.float32)
    return xs_cols, ysb, ident16


def _build():
    import concourse.bacc as bacc
    import concourse.bass as bass
    import concourse.mybir as mybir
    from concourse.tile import TileContext

    A = mybir.AluOpType
    F = mybir.ActivationFunctionType
    f32 = mybir.dt.float32

    nc = bacc.Bacc("TRN2", target_bir_lowering=False, debug=False)

    image = nc.dram_tensor("image", [B, NX, NY], f32, kind="ExternalInput")
    traj = nc.dram_tensor("traj", [2, ML], f32, kind="ExternalInput")
    xs_cols = nc.dram_tensor("xs_cols", [128, 2], f32, kind="ExternalInput")
    ysb = nc.dram_tensor("ysb", [128, NSEG], f32, kind="ExternalInput")
    ident16 = nc.dram_tensor("ident16", [16, 16], f32, kind="ExternalInput")
    out = nc.dram_tensor("out", [128, 4 * NT], f32, kind="ExternalOutput")

    mmdt = mybir.dt.float32r if USE_F32R else f32

    def mmcast(ap):
        return ap

    def seg2(ap_tile, start, seg_stride):
        """[128, 2, NS] view: two NS-wide segments at start, start+seg_stride."""
        t_ = ap_tile.tensor
        row = ap_tile.ap[0][0]
        return bass.AP(t_, ap_tile.offset + start,
                       [[row, 128], [seg_stride, 2], [1, NS]])

    with TileContext(nc) as tc:
        with tc.tile_pool(name="const", bufs=1) as cpool, \
             tc.tile_pool(name="xtab", bufs=1) as xpool, \
             tc.tile_pool(name="xscratch", bufs=2) as xs_pool, \
             tc.tile_pool(name="ytab", bufs=10) as ypool, \
             tc.tile_pool(name="work", bufs=6) as wpool:

            # ---------------- constants / inputs ----------------
            xs_sb = cpool.tile([128, 2], f32)
            nc.sync.dma_start(xs_sb[:, :], xs_cols[:, :])
            kxb = cpool.tile([128, ML], f32)
            nc.sync.dma_start(kxb[:, :], traj[0:1, :].to_broadcast((128, ML)))
            ysb_sb = cpool.tile([128, NSEG], f32)
            nc.sync.dma_start(ysb_sb[:, :], ysb[:, :])
            id16 = cpool.tile([16, 16], f32)
            nc.sync.dma_start(id16[:, :], ident16[:, :])

            ky16 = cpool.tile([16, 128], f32)
            nc.sync.dma_start(
                ky16[:, :], traj[1:2, :].rearrange("o (t p) -> (o t) p", p=128))
            ky_col = cpool.tile([128, NT], f32)
            half_pi = cpool.tile([128, 1], f32)
            nc.vector.memset(half_pi[:, :], PI / 2.0)

            # image load + even/odd y-fold into concat rhs [odd | even]
            img_oe = {}
            for b in range(B):
                for k in range(2):
                    raw = wpool.tile([128, NY], f32, tag="imgraw")
                    nc.sync.dma_start(
                        raw[:, :], image[b, k * 128:(k + 1) * 128, :])
                    oe = cpool.tile([128, NSEG], mmdt, name=f"ioe_{b}_{k}")
                    # odd seg: cols 0:127 pairs, 127 = img[:,0], 128 = zero pad
                    nc.vector.tensor_sub(
                        oe[:, 0:127], raw[:, 129:256], raw[:, 127:0:-1])
                    nc.scalar.copy(oe[:, 127:128], raw[:, 0:1])
                    nc.scalar.mul(oe[:, 128:129], raw[:, 0:1], 0.0)
                    # even seg: col 129 = img[:,128], 130:257 pairs, 257 = img[:,0]
                    nc.scalar.copy(oe[:, 129:130], raw[:, 128:129])
                    nc.vector.tensor_add(
                        oe[:, 130:257], raw[:, 129:256], raw[:, 127:0:-1])
                    nc.scalar.copy(oe[:, 257:258], raw[:, 0:1])
                    img_oe[(b, k)] = oe

            # ---------------- x tables: CxT/SxT [x(2x128), m(2048)] --------
            cxT = [xpool.tile([128, ML], mmdt, name=f"cxT{h}") for h in range(2)]
            sxT = [xpool.tile([128, ML], mmdt, name=f"sxT{h}") for h in range(2)]
            with tc.tile_pool(name="psP", bufs=1, space="PSUM") as psP:
                ky_ps = psP.tile([128, 16], f32, tag="kyT")
                nc.tensor.transpose(ky_ps[:, :], ky16[:, :], id16[:, :])
                nc.scalar.copy(ky_col[:, :], ky_ps[:, :])
            for h in range(2):
                P = xs_pool.tile([128, ML], f32, tag="xP")
                rs = xs_pool.tile([128, ML], f32, tag="xrs")
                fs = xs_pool.tile([128, ML], f32, tag="xfs")
                fa = xs_pool.tile([128, ML], f32, tag="xfa")
                for j in range(2):
                    js = slice(j * 1024, (j + 1) * 1024)
                    nc.gpsimd.tensor_scalar(
                        P[:, js], kxb[:, js], scalar1=xs_sb[:, h:h + 1],
                        scalar2=None, op0=A.mult)
                    nc.gpsimd.tensor_scalar(
                        rs[:, js], P[:, js], scalar1=MAGIC, scalar2=MAGIC,
                        op0=A.add, op1=A.subtract)
                    nc.vector.scalar_tensor_tensor(
                        fs[:, js], P[:, js], 1.0, rs[:, js],
                        op0=A.mult, op1=A.subtract)
                    nc.scalar.activation(
                        sxT[h][:, js], fs[:, js], F.Sin, scale=TWO_PI)
                    nc.scalar.activation(fa[:, js], fs[:, js], F.Abs)
                    nc.scalar.activation(
                        cxT[h][:, js], fa[:, js], F.Sin, scale=-TWO_PI,
                        bias=half_pi[:, :])

            # ---------------- per m-tile main loop ----------------
            out_sb = cpool.tile([128, 4 * NT], f32)
            psAB_cm = tc.tile_pool(name="psAB", bufs=4, space="PSUM")
            psAB = psAB_cm.__enter__()
            ytiles = {}
            def gen_w(tv):
                t = tv
                # --- shared y table W = [-Sy'(129) | Cy(129) | Sy'(129)] ---
                u = ky_col[:, t:t + 1]
                p_y = ypool.tile([128, NSEG], f32, tag="py")
                nc.gpsimd.tensor_scalar(
                    p_y[:, :], ysb_sb[:, :], scalar1=u, scalar2=None, op0=A.mult)
                rs_y = ypool.tile([128, NSEG], f32, tag="yrs")
                nc.gpsimd.tensor_scalar(
                    rs_y[:, :], p_y[:, :], scalar1=MAGIC, scalar2=MAGIC,
                    op0=A.add, op1=A.subtract)
                fs_y = ypool.tile([128, NSEG], f32, tag="yfs")
                nc.vector.scalar_tensor_tensor(
                    fs_y[:, :], p_y[:, :], 1.0, rs_y[:, :],
                    op0=A.mult, op1=A.subtract)
                w = ypool.tile([128, NW], f32, tag="w")
                nc.scalar.activation(
                    w[:, 0:NS], fs_y[:, 0:NS], F.Sin, scale=-TWO_PI)
                nc.scalar.activation(
                    w[:, 2 * NS:NW], fs_y[:, 0:NS], F.Sin, scale=TWO_PI)
                fa_y = ypool.tile([128, NS], f32, tag="yfa")
                nc.scalar.activation(fa_y[:, :], fs_y[:, NS:NSEG], F.Abs)
                nc.scalar.activation(
                    w[:, NS:2 * NS], fa_y[:, :], F.Sin, scale=-TWO_PI,
                    bias=half_pi[:, :])

                ytiles[tv] = w
            gen_w(0)
            gen_w(1)
            for t in range(NT):
                if t + 2 < NT:
                    gen_w(t + 2)
                wt = ytiles.pop(t)
                for b in range(B):
                    # --- stage 1: bank0 = [B_odd|B_even], bank1 = [A_odd|A_even]
                    ab = psAB.tile([128, 1024], f32, tag="ab")
                    for k in range(2):
                        nc.tensor.matmul(
                            ab[:, 0:NSEG],
                            mmcast(sxT[k][:, t * 128:(t + 1) * 128]),
                            mmcast(img_oe[(b, k)][:, :]),
                            start=(k == 0), stop=(k == 1))
                    for k in range(2):
                        nc.tensor.matmul(
                            ab[:, 512:512 + NSEG],
                            mmcast(cxT[k][:, t * 128:(t + 1) * 128]),
                            mmcast(img_oe[(b, k)][:, :]),
                            start=(k == 0), stop=(k == 1))
                    # --- stage 2: fused multiply + row-reduce (strided APs) ---
                    # Re  = sum(B_odd * -Sy') + sum(A_even * Cy)
                    #       in0 segments at col 0 (B_odd), col 512+129 (A_even)
                    # -Im = sum(B_even * Cy) + sum(A_odd * Sy')
                    #       in0 segments at col 129 (B_even), col 512 (A_odd)
                    scr = wpool.tile([128, NSEG], f32, tag="scr")
                    scr2 = wpool.tile([128, NSEG], f32, tag="scr2")
                    col_re = (2 * b) * NT + t
                    col_im = (2 * b + 1) * NT + t
                    nc.vector.scalar_tensor_tensor(
                        seg2(scr, 0, NS), seg2(ab, 0, 641), 1.0,
                        seg2(wt, 0, NS),
                        op0=A.mult, op1=A.mult,
                        accum_out=out_sb[:, col_re:col_re + 1])
                    nc.vector.scalar_tensor_tensor(
                        seg2(scr2, 0, NS), seg2(ab, NS, 383), 1.0,
                        seg2(wt, NS, NS),
                        op0=A.mult, op1=A.mult,
                        accum_out=out_sb[:, col_im:col_im + 1])

            nc.sync.dma_start(out[:, :], out_sb[:, :])
            psAB_cm.__exit__(None, None, None)

    nc.compile()
    return nc


def kernel(image, trajectory):
    from concourse.bass_utils import run_bass_kernel_spmd

    if 'nc' not in _CACHE:
        _CACHE['nc'] = _build()
    nc = _CACHE['nc']

    image = np.ascontiguousarray(np.asarray(image, dtype=np.float32))
    trajectory = np.ascontiguousarray(np.asarray(trajectory, dtype=np.float32))
    xs_cols, ysb, ident16 = _consts()

    in_maps = []
    for c in range(NCORES):
        in_maps.append({
            "image": image,
            "traj": np.ascontiguousarray(trajectory[:, c * ML:(c + 1) * ML]),
            "xs_cols": xs_cols,
            "ysb": ysb,
            "ident16": ident16,
        })

    res = run_bass_kernel_spmd(nc, in_maps, core_ids=list(range(NCORES)))

    kspace = np.empty((B, M), dtype=np.complex64)
    for c in range(NCORES):
        o = res.results[c]["out"]          # [128, 4*NT]
        o = o.reshape(128, 2, 2, NT)       # [p, b, (re, -im), t]
        for b in range(B):
            re = o[:, b, 0, :].T.reshape(ML)   # m = t*128 + p
            im = -o[:, b, 1, :].T.reshape(ML)
            kspace[b, c * ML:(c + 1) * ML] = re + 1j * im
    return kspace



# revision 4
# speedup vs baseline: 1.7399x; 1.7399x over previous
"""Type-2 NUFFT (image -> non-uniform k-space) on 8 Trainium2 NeuronCores.

kspace[b,m] = sum_{x,y} image[b,x,y] * exp(-i*(kx_m*(x-128) + ky_m*(y-128)))

Quarter-fold decomposition with half-integer centering: write
x-128 = v - 1/2 with v = x - 127.5 in +-{0.5, ..., 127.5}, and likewise
y-128 = u - 1/2.  Then

  kspace[b,m] = e^{i(kx+ky)/2} * sum_{v,u} img * e^{-i(kx v + ky u)}

and the inner sum folds EXACTLY into 128x128 quadrant images (cos is even,
sin is odd in both v and u):

  inner = R - i*N
  R[m] = sum_w C~E[m,w]*cosY[m,w] - S~O[m,w]*sinY[m,w]
  N[m] = sum_w C~O[m,w]*sinY[m,w] + S~E[m,w]*cosY[m,w]
  C~E = cosX^T @ imgEE   C~O = cosX^T @ imgEO      (per batch)
  S~E = sinX^T @ imgOE   S~O = sinX^T @ imgOO

Work split: the host (numpy) computes the trig tables (cosX/sinX [128,2048]
per core, cosY/sinY per m-tile) and the folded quadrant images in bf16, and
applies the final e^{i(kx+ky)/2} rotation.  The device does the O(M*N^2)
work only: per (batch, m-tile) two bf16 matmuls into PSUM and two fused
multiply+row-reduce ops (DVE for b=0, GPSIMD for b=1 so they run
concurrently), accumulating straight into the output column.
"""

import sys

if '/opt/trn_rl_repo' not in sys.path:
    sys.path.insert(0, '/opt/trn_rl_repo')

import numpy as np
import ml_dtypes

B, NX, NY, M, NCORES = 2, 256, 256, 16384, 8
ML = M // NCORES            # 2048 m-points per core
NT = ML // 128              # 16 m-tiles per core

_CACHE = {}


def _build():
    import concourse.bacc as bacc
    import concourse.bass as bass
    import concourse.mybir as mybir
    from concourse.tile import TileContext

    A = mybir.AluOpType
    f32 = mybir.dt.float32
    bf16 = mybir.dt.bfloat16

    nc = bacc.Bacc("TRN2", target_bir_lowering=False, debug=False)

    imgq = nc.dram_tensor("imgq", [B, 128, 512], bf16, kind="ExternalInput")
    cxt = nc.dram_tensor("cxt", [128, ML], bf16, kind="ExternalInput")
    sxt = nc.dram_tensor("sxt", [128, ML], bf16, kind="ExternalInput")
    wtab = nc.dram_tensor("wtab", [128, NT * 256], bf16, kind="ExternalInput")
    out = nc.dram_tensor("out", [128, 4 * NT], f32, kind="ExternalOutput")

    def seg2(tile_ap, start, seg_stride):
        """[128, 2, 128] view: two 128-wide segments at start, start+stride."""
        t_ = tile_ap.tensor
        row = tile_ap.ap[0][0]
        return bass.AP(t_, tile_ap.offset + start,
                       [[row, 128], [seg_stride, 2], [1, 128]])

    NCH = 4                     # DMA chunks over the m dimension
    TCH = NT // NCH             # t-tiles per chunk

    with TileContext(nc) as tc:
        with tc.tile_pool(name="const", bufs=1) as cpool, \
             tc.tile_pool(name="work", bufs=4) as wpool, \
             tc.tile_pool(name="ps", bufs=4, space="PSUM") as ps:

            img_sb = []
            for b in range(B):
                ib = cpool.tile([128, 512], bf16, name=f"img{b}")
                img_sb.append(ib)
            cx_sb = cpool.tile([128, ML], bf16, name="cx")
            sx_sb = cpool.tile([128, ML], bf16, name="sx")
            w_sb = cpool.tile([128, NT * 256], bf16, name="w")
            out_sb = cpool.tile([128, 4 * NT], f32)

            # chunked loads, interleaved so t=0 compute starts early
            for c in range(NCH):
                ms = slice(c * (ML // NCH), (c + 1) * (ML // NCH))
                nc.sync.dma_start(cx_sb[:, ms], cxt[:, ms])
                nc.sync.dma_start(sx_sb[:, ms], sxt[:, ms])
                if c == 0:
                    for b in range(B):
                        nc.sync.dma_start(img_sb[b][:, :], imgq[b, :, :])
                ws = slice(c * TCH * 256, (c + 1) * TCH * 256)
                nc.sync.dma_start(w_sb[:, ws], wtab[:, ws])

            for t in range(NT):
                xsl = slice(t * 128, (t + 1) * 128)
                w0 = t * 256
                for b in range(B):
                    ab = ps.tile([128, 512], f32, tag="ab")
                    # ab = [C~E | C~O | S~E | -S~O]
                    nc.tensor.matmul(ab[:, 0:256], cx_sb[:, xsl],
                                     img_sb[b][:, 0:256],
                                     start=True, stop=True)
                    nc.tensor.matmul(ab[:, 256:512], sx_sb[:, xsl],
                                     img_sb[b][:, 256:512],
                                     start=True, stop=True)
                    eng, src = nc.vector, ab
                    scr = wpool.tile([128, 256], f32, tag=f"scr{b}")
                    scr2 = wpool.tile([128, 256], f32, tag=f"scr2_{b}")
                    col_re = (2 * b) * NT + t
                    col_im = (2 * b + 1) * NT + t
                    # R = sum(C~E*cosY) + sum(-S~O*sinY)
                    eng.scalar_tensor_tensor(
                        seg2(scr[:, :], 0, 128),
                        seg2(src[:, :], 0, 384), 1.0,
                        seg2(w_sb[:, :], w0, 128),
                        op0=A.mult, op1=A.mult,
                        accum_out=out_sb[:, col_re:col_re + 1])
                    # N = sum(C~O*sinY) + sum(S~E*cosY)
                    eng.scalar_tensor_tensor(
                        seg2(scr2[:, :], 0, 128),
                        seg2(src[:, :], 128, 128), 1.0,
                        seg2(w_sb[:, :], w0 + 128, -128),
                        op0=A.mult, op1=A.mult,
                        accum_out=out_sb[:, col_im:col_im + 1])

            nc.sync.dma_start(out[:, :], out_sb[:, :])

    nc.compile()
    return nc


def _host_prep(image, trajectory):
    """Folded quadrant images, trig tables (bf16) and the phase, per core."""
    bf = ml_dtypes.bfloat16
    kx = trajectory[0].astype(np.float32)            # [M]
    ky = trajectory[1].astype(np.float32)
    v = (np.arange(128, dtype=np.float32) + 0.5)

    cosX = np.cos(kx[None, :] * v[:, None])          # [128, M]
    sinX = np.sin(kx[None, :] * v[:, None])
    argY = ky[:, None] * v[None, :]                  # [M, 128]
    cosY = np.cos(argY)
    sinY = np.sin(argY)

    # wtab[core][p, t*256 + s] = [cosY | sinY](m = core*ML + t*128 + p, w)
    cy = cosY.reshape(NCORES, NT, 128, 128)
    sy = sinY.reshape(NCORES, NT, 128, 128)
    wt = np.concatenate([cy, sy], axis=-1)           # [C, T, p, 256]
    wtab = np.ascontiguousarray(wt.transpose(0, 2, 1, 3)
                                .reshape(NCORES, 128, NT * 256)).astype(bf)

    cxt = np.ascontiguousarray(
        cosX.reshape(128, NCORES, ML).transpose(1, 0, 2)).astype(bf)
    sxt = np.ascontiguousarray(
        sinX.reshape(128, NCORES, ML).transpose(1, 0, 2)).astype(bf)

    # quadrant folds (x: rows about 127.5; y: cols about 127.5)
    top = image[:, 128:256, :]
    bot = image[:, 127::-1, :]
    sumx = top + bot
    difx = top - bot
    imgEE = sumx[:, :, 128:256] + sumx[:, :, 127::-1]
    imgEO = sumx[:, :, 128:256] - sumx[:, :, 127::-1]
    imgOE = difx[:, :, 128:256] + difx[:, :, 127::-1]
    imgOOn = difx[:, :, 127::-1] - difx[:, :, 128:256]   # = -imgOO
    imgq = np.concatenate([imgEE, imgEO, imgOE, imgOOn], axis=2).astype(bf)

    phase = np.exp(1j * (kx + ky) / 2.0).astype(np.complex64)
    return imgq, cxt, sxt, wtab, phase


def kernel(image, trajectory):
    from concourse.bass_utils import run_bass_kernel_spmd

    if 'nc' not in _CACHE:
        _CACHE['nc'] = _build()
    nc = _CACHE['nc']

    image = np.ascontiguousarray(np.asarray(image, dtype=np.float32))
    trajectory = np.ascontiguousarray(np.asarray(trajectory, dtype=np.float32))
    imgq, cxt, sxt, wtab, phase = _host_prep(image, trajectory)

    in_maps = []
    for c in range(NCORES):
        in_maps.append({
            "imgq": imgq,
            "cxt": np.ascontiguousarray(cxt[c]),
            "sxt": np.ascontiguousarray(sxt[c]),
            "wtab": np.ascontiguousarray(wtab[c]),
        })

    res = run_bass_kernel_spmd(nc, in_maps, core_ids=list(range(NCORES)))

    kspace = np.empty((B, M), dtype=np.complex64)
    for c in range(NCORES):
        o = res.results[c]["out"]          # [128, 4*NT]
        o = o.reshape(128, 2, 2, NT)       # [p, b, (R, N), t]
        for b in range(B):
            R = o[:, b, 0, :].T.reshape(ML)    # m = t*128 + p
            N = o[:, b, 1, :].T.reshape(ML)
            kspace[b, c * ML:(c + 1) * ML] = R - 1j * N
    kspace *= phase[None, :]
    return kspace


# revision 6
# speedup vs baseline: 1.8234x; 1.0480x over previous
"""Type-2 NUFFT (image -> non-uniform k-space) on 8 Trainium2 NeuronCores.

kspace[b,m] = sum_{x,y} image[b,x,y] * exp(-i*(kx_m*(x-128) + ky_m*(y-128)))

Quarter-fold decomposition with half-integer centering: write
x-128 = v - 1/2 with v = x - 127.5 in +-{0.5, ..., 127.5}, and likewise
y-128 = u - 1/2.  Then

  kspace[b,m] = e^{i(kx+ky)/2} * sum_{v,u} img * e^{-i(kx v + ky u)}

and the inner sum folds EXACTLY into 128x128 quadrant images (cos is even,
sin is odd in both v and u):

  inner = R - i*N
  R[m] = sum_w C~E[m,w]*cosY[m,w] - S~O[m,w]*sinY[m,w]
  N[m] = sum_w C~O[m,w]*sinY[m,w] + S~E[m,w]*cosY[m,w]
  C~E = cosX^T @ imgEE   C~O = cosX^T @ imgEO      (per batch)
  S~E = sinX^T @ imgOE   S~O = sinX^T @ imgOO

Work split: the host (numpy) computes the trig tables (cosX/sinX per m-tile,
cosY/sinY per m-tile) and the folded quadrant images in bf16, packs them
into one consumption-ordered blob, and applies the final e^{i(kx+ky)/2}
rotation.  The device does the O(M*N^2) work only: per (batch, m-tile) two
bf16 matmuls into PSUM and two fused DVE multiply+row-reduce ops that
accumulate straight into the output column.
"""

import sys

if '/opt/trn_rl_repo' not in sys.path:
    sys.path.insert(0, '/opt/trn_rl_repo')

import numpy as np
import ml_dtypes

B, NX, NY, M, NCORES = 2, 256, 256, 16384, 8
ML = M // NCORES            # 2048 m-points per core
NT = ML // 128              # 16 m-tiles per core

# blob layout (bf16, per partition-col): [img b0 (512) | img b1 (512) |
#   per-t: cx(128) | sx(128) | w=cosY|sinY (256)]  => 1024 + NT*512 cols
IMG0 = 0
TBL0 = 1024
TSTRIDE = 512
BLOB_COLS = TBL0 + NT * TSTRIDE

_CACHE = {}


def _build():
    import concourse.bacc as bacc
    import concourse.bass as bass
    import concourse.mybir as mybir
    from concourse.tile import TileContext

    A = mybir.AluOpType
    f32 = mybir.dt.float32
    bf16 = mybir.dt.bfloat16

    nc = bacc.Bacc("TRN2", target_bir_lowering=False, debug=False)

    blob = nc.dram_tensor("blob", [128, BLOB_COLS], bf16, kind="ExternalInput")
    out = nc.dram_tensor("out", [128, 4 * NT], f32, kind="ExternalOutput")

    def seg2(tile_ap, start, seg_stride):
        """[128, 2, 128] view: two 128-wide segments at start, start+stride."""
        t_ = tile_ap.tensor
        row = tile_ap.ap[0][0]
        return bass.AP(t_, tile_ap.offset + start,
                       [[row, 128], [seg_stride, 2], [1, 128]])

    # DMA chunks over the blob, in consumption order (first small, for a
    # fast pipeline start)
    bounds = [0, TBL0 + 1 * TSTRIDE, TBL0 + 3 * TSTRIDE, TBL0 + 7 * TSTRIDE,
              TBL0 + 11 * TSTRIDE, BLOB_COLS]

    with TileContext(nc) as tc:
        with tc.tile_pool(name="const", bufs=1) as cpool, \
             tc.tile_pool(name="work", bufs=4) as wpool, \
             tc.tile_pool(name="ps", bufs=4, space="PSUM") as ps:

            bsb = cpool.tile([128, BLOB_COLS], bf16, name="blob")
            out_sb = cpool.tile([128, 4 * NT], f32)

            for i in range(len(bounds) - 1):
                cs = slice(bounds[i], bounds[i + 1])
                nc.sync.dma_start(bsb[:, cs], blob[:, cs])

            for t in range(NT):
                c0 = TBL0 + t * TSTRIDE
                for b in range(B):
                    ab = ps.tile([128, 512], f32, tag="ab")
                    # ab = [C~E | C~O | S~E | -S~O]
                    nc.tensor.matmul(ab[:, 0:256],
                                     bsb[:, c0:c0 + 128],
                                     bsb[:, b * 512:b * 512 + 256],
                                     start=True, stop=True)
                    nc.tensor.matmul(ab[:, 256:512],
                                     bsb[:, c0 + 128:c0 + 256],
                                     bsb[:, b * 512 + 256:b * 512 + 512],
                                     start=True, stop=True)
                    scr = wpool.tile([128, 256], f32, tag="scr")
                    scr2 = wpool.tile([128, 256], f32, tag="scr2")
                    col = t * 4 + b * 2
                    # R = sum(C~E*cosY) + sum(-S~O*sinY)
                    nc.vector.scalar_tensor_tensor(
                        seg2(scr[:, :], 0, 128),
                        seg2(ab[:, :], 0, 384), 1.0,
                        seg2(bsb[:, :], c0 + 256, 128),
                        op0=A.mult, op1=A.mult,
                        accum_out=out_sb[:, col:col + 1])
                    # N = sum(C~O*sinY) + sum(S~E*cosY)
                    nc.vector.scalar_tensor_tensor(
                        seg2(scr2[:, :], 0, 128),
                        seg2(ab[:, :], 128, 128), 1.0,
                        seg2(bsb[:, :], c0 + 384, -128),
                        op0=A.mult, op1=A.mult,
                        accum_out=out_sb[:, col + 1:col + 2])
                if t % 4 == 3:
                    qs = slice((t - 3) * 4, (t + 1) * 4)
                    nc.sync.dma_start(out[:, qs], out_sb[:, qs])

    nc.compile()
    return nc


def _host_prep(image, trajectory):
    """Folded quadrant images + trig tables (bf16) packed per-core blobs."""
    bf = ml_dtypes.bfloat16
    kx = trajectory[0].astype(np.float32)            # [M]
    ky = trajectory[1].astype(np.float32)
    v = (np.arange(128, dtype=np.float32) + 0.5)

    cosX = np.cos(kx[None, :] * v[:, None])          # [128, M]
    sinX = np.sin(kx[None, :] * v[:, None])
    argY = ky[:, None] * v[None, :]                  # [M, 128]
    cosY = np.cos(argY)
    sinY = np.sin(argY)

    # quadrant folds (x: rows about 127.5; y: cols about 127.5)
    top = image[:, 128:256, :]
    bot = image[:, 127::-1, :]
    sumx = top + bot
    difx = top - bot
    imgEE = sumx[:, :, 128:256] + sumx[:, :, 127::-1]
    imgEO = sumx[:, :, 128:256] - sumx[:, :, 127::-1]
    imgOE = difx[:, :, 128:256] + difx[:, :, 127::-1]
    imgOOn = difx[:, :, 127::-1] - difx[:, :, 128:256]   # = -imgOO
    imgq = np.concatenate([imgEE, imgEO, imgOE, imgOOn], axis=2)  # [B,128,512]

    # per-core blob [128, BLOB_COLS]
    cx = cosX.reshape(128, NCORES, NT, 128)          # [j, c, t, p]
    sx = sinX.reshape(128, NCORES, NT, 128)
    cy = cosY.reshape(NCORES, NT, 128, 128)          # [c, t, p, w]
    sy = sinY.reshape(NCORES, NT, 128, 128)

    blobs = np.empty((NCORES, 128, BLOB_COLS), dtype=bf)
    blobs[:, :, 0:512] = imgq[0].astype(bf)[None]
    blobs[:, :, 512:1024] = imgq[1].astype(bf)[None]
    # tables: for core c, tile t: cols [cx_t | sx_t | cy_t | sy_t]
    tbl = np.concatenate([
        cx.transpose(1, 2, 0, 3),                    # [c, t, j, p] -> cx block
        sx.transpose(1, 2, 0, 3),
        cy.transpose(0, 1, 2, 3),                    # [c, t, p, w]
        sy.transpose(0, 1, 2, 3),
    ], axis=-1)                                       # [c, t, 128, 512]
    blobs[:, :, TBL0:] = tbl.transpose(0, 2, 1, 3).reshape(
        NCORES, 128, NT * TSTRIDE).astype(bf)

    phase = np.exp(1j * (kx + ky) / 2.0).astype(np.complex64)
    return blobs, phase


def kernel(image, trajectory):
    from concourse.bass_utils import run_bass_kernel_spmd

    if 'nc' not in _CACHE:
        _CACHE['nc'] = _build()
    nc = _CACHE['nc']

    image = np.ascontiguousarray(np.asarray(image, dtype=np.float32))
    trajectory = np.ascontiguousarray(np.asarray(trajectory, dtype=np.float32))
    blobs, phase = _host_prep(image, trajectory)

    in_maps = [{"blob": np.ascontiguousarray(blobs[c])} for c in range(NCORES)]

    res = run_bass_kernel_spmd(nc, in_maps, core_ids=list(range(NCORES)))

    kspace = np.empty((B, M), dtype=np.complex64)
    for c in range(NCORES):
        o = res.results[c]["out"]          # [128, 4*NT]
        o = o.reshape(128, NT, 2, 2)       # [p, t, b, (R, N)]
        for b in range(B):
            R = o[:, :, b, 0].T.reshape(ML)    # m = t*128 + p
            N = o[:, :, b, 1].T.reshape(ML)
            kspace[b, c * ML:(c + 1) * ML] = R - 1j * N
    kspace *= phase[None, :]
    return kspace


# revision 8
# speedup vs baseline: 1.9854x; 1.0889x over previous
"""Type-2 NUFFT (image -> non-uniform k-space) on 8 Trainium2 NeuronCores.

kspace[b,m] = sum_{x,y} image[b,x,y] * exp(-i*(kx_m*(x-128) + ky_m*(y-128)))

Quarter-fold decomposition with half-integer centering: write
x-128 = v - 1/2 with v = x - 127.5 in +-{0.5, ..., 127.5}, and likewise
y-128 = u - 1/2.  Then

  kspace[b,m] = e^{i(kx+ky)/2} * sum_{v,u} img * e^{-i(kx v + ky u)}

and the inner sum folds EXACTLY into 128x128 quadrant images (cos is even,
sin is odd in both v and u):

  inner = R - i*N
  R[m] = sum_w C~E[m,w]*cosY[m,w] - S~O[m,w]*sinY[m,w]
  N[m] = sum_w C~O[m,w]*sinY[m,w] + S~E[m,w]*cosY[m,w]
  C~E = cosX^T @ imgEE   C~O = cosX^T @ imgEO      (per batch)
  S~E = sinX^T @ imgOE   S~O = sinX^T @ imgOO

Work split: the host (numpy) computes the trig tables (cosX/sinX per m-tile,
cosY/sinY per m-tile) and the folded quadrant images in bf16, packs them
into one consumption-ordered blob, and applies the final e^{i(kx+ky)/2}
rotation.  The device does the O(M*N^2) work only: per (batch, m-tile) two
bf16 matmuls into PSUM and two fused DVE multiply+row-reduce ops that
accumulate straight into the output column.
"""

import sys

if '/opt/trn_rl_repo' not in sys.path:
    sys.path.insert(0, '/opt/trn_rl_repo')

import numpy as np
import ml_dtypes

B, NX, NY, M, NCORES = 2, 256, 256, 16384, 8
ML = M // NCORES            # 2048 m-points per core
NT = ML // 128              # 16 m-tiles per core

# blob layout (bf16, per partition-col): [img b0 (512) | img b1 (512) |
#   per-t: cx(128) | sx(128) | w=cosY|sinY (256)]  => 1024 + NT*512 cols
IMG0 = 0
TBL0 = 1024
TSTRIDE = 512
BLOB_COLS = TBL0 + NT * TSTRIDE

_CACHE = {}


def _build():
    import concourse.bacc as bacc
    import concourse.bass as bass
    import concourse.mybir as mybir
    from concourse.tile import TileContext

    A = mybir.AluOpType
    f32 = mybir.dt.float32
    bf16 = mybir.dt.bfloat16

    nc = bacc.Bacc("TRN2", target_bir_lowering=False, debug=False)

    blob = nc.dram_tensor("blob", [128, BLOB_COLS], bf16, kind="ExternalInput")
    out = nc.dram_tensor("out", [128, 4 * NT], f32, kind="ExternalOutput")

    def seg2(tile_ap, start, seg_stride):
        """[128, 2, 128] view: two 128-wide segments at start, start+stride."""
        t_ = tile_ap.tensor
        row = tile_ap.ap[0][0]
        return bass.AP(t_, tile_ap.offset + start,
                       [[row, 128], [seg_stride, 2], [1, 128]])

    # DMA chunks over the blob, in consumption order (first small, for a
    # fast pipeline start)
    bounds = [0, TBL0 + 1 * TSTRIDE, TBL0 + 3 * TSTRIDE, TBL0 + 7 * TSTRIDE,
              TBL0 + 11 * TSTRIDE, BLOB_COLS]

    with TileContext(nc) as tc:
        with tc.tile_pool(name="const", bufs=1) as cpool, \
             tc.tile_pool(name="work", bufs=4) as wpool, \
             tc.tile_pool(name="ps", bufs=4, space="PSUM") as ps:

            bsb = cpool.tile([128, BLOB_COLS], bf16, name="blob")
            out_sb = cpool.tile([128, 4 * NT], f32)

            for i in range(len(bounds) - 1):
                cs = slice(bounds[i], bounds[i + 1])
                nc.sync.dma_start(bsb[:, cs], blob[:, cs])

            # per-(t,b) stage-2 path: A = DVE fused multiply+reduce from PSUM;
            # B = Act evicts PSUM->SBUF, Pool multiplies, DVE reduces (2x);
            # C = like B but Act reduces.  Balances DVE/Act/Pool busy time.
            PATTERN = "ACAABACA" * 4
            F = mybir.ActivationFunctionType

            for t in range(NT):
                c0 = TBL0 + t * TSTRIDE
                for b in range(B):
                    ab = ps.tile([128, 512], f32, tag="ab")
                    # ab = [C~E | C~O | S~E | -S~O]
                    nc.tensor.matmul(ab[:, 0:256],
                                     bsb[:, c0:c0 + 128],
                                     bsb[:, b * 512:b * 512 + 256],
                                     start=True, stop=True)
                    nc.tensor.matmul(ab[:, 256:512],
                                     bsb[:, c0 + 128:c0 + 256],
                                     bsb[:, b * 512 + 256:b * 512 + 512],
                                     start=True, stop=True)
                    col = t * 4 + b * 2
                    path = PATTERN[t * 2 + b]
                    # R = sum(C~E*cosY) + sum(-S~O*sinY)   -> col
                    # N = sum(C~O*sinY) + sum(S~E*cosY)    -> col+1
                    if path == 'A':
                        scr = wpool.tile([128, 256], f32, tag="scr")
                        scr2 = wpool.tile([128, 256], f32, tag="scr2")
                        nc.vector.scalar_tensor_tensor(
                            seg2(scr[:, :], 0, 128),
                            seg2(ab[:, :], 0, 384), 1.0,
                            seg2(bsb[:, :], c0 + 256, 128),
                            op0=A.mult, op1=A.mult,
                            accum_out=out_sb[:, col:col + 1])
                        nc.vector.scalar_tensor_tensor(
                            seg2(scr2[:, :], 0, 128),
                            seg2(ab[:, :], 128, 128), 1.0,
                            seg2(bsb[:, :], c0 + 384, -128),
                            op0=A.mult, op1=A.mult,
                            accum_out=out_sb[:, col + 1:col + 2])
                    else:
                        cp = wpool.tile([128, 512], f32, tag="cp")
                        nc.scalar.copy(cp[:, :], ab[:, :])
                        p1 = wpool.tile([128, 256], f32, tag="p1")
                        p2 = wpool.tile([128, 256], f32, tag="p2")
                        nc.gpsimd.tensor_tensor(
                            seg2(p1[:, :], 0, 128),
                            seg2(cp[:, :], 0, 384),
                            seg2(bsb[:, :], c0 + 256, 128), op=A.mult)
                        nc.gpsimd.tensor_tensor(
                            seg2(p2[:, :], 0, 128),
                            seg2(cp[:, :], 128, 128),
                            seg2(bsb[:, :], c0 + 384, -128), op=A.mult)
                        if path == 'B':
                            d1 = wpool.tile([128, 256], f32, tag="d1")
                            d2 = wpool.tile([128, 256], f32, tag="d2")
                            nc.vector.tensor_scalar(
                                d1[:, :], p1[:, :], scalar1=1.0, scalar2=0.0,
                                op0=A.mult, op1=A.add,
                                accum_out=out_sb[:, col:col + 1])
                            nc.vector.tensor_scalar(
                                d2[:, :], p2[:, :], scalar1=1.0, scalar2=0.0,
                                op0=A.mult, op1=A.add,
                                accum_out=out_sb[:, col + 1:col + 2])
                        else:
                            d1 = wpool.tile([128, 256], f32, tag="d1")
                            d2 = wpool.tile([128, 256], f32, tag="d2")
                            nc.scalar.activation(
                                d1[:, :], p1[:, :], F.Copy,
                                accum_out=out_sb[:, col:col + 1])
                            nc.scalar.activation(
                                d2[:, :], p2[:, :], F.Copy,
                                accum_out=out_sb[:, col + 1:col + 2])
                if t % 4 == 3:
                    qs = slice((t - 3) * 4, (t + 1) * 4)
                    nc.sync.dma_start(out[:, qs], out_sb[:, qs])

    nc.compile()
    return nc


def _host_prep(image, trajectory):
    """Folded quadrant images + trig tables (bf16) packed per-core blobs."""
    bf = ml_dtypes.bfloat16
    kx = trajectory[0].astype(np.float32)            # [M]
    ky = trajectory[1].astype(np.float32)
    v = (np.arange(128, dtype=np.float32) + 0.5)

    cosX = np.cos(kx[None, :] * v[:, None])          # [128, M]
    sinX = np.sin(kx[None, :] * v[:, None])
    argY = ky[:, None] * v[None, :]                  # [M, 128]
    cosY = np.cos(argY)
    sinY = np.sin(argY)

    # quadrant folds (x: rows about 127.5; y: cols about 127.5)
    top = image[:, 128:256, :]
    bot = image[:, 127::-1, :]
    sumx = top + bot
    difx = top - bot
    imgEE = sumx[:, :, 128:256] + sumx[:, :, 127::-1]
    imgEO = sumx[:, :, 128:256] - sumx[:, :, 127::-1]
    imgOE = difx[:, :, 128:256] + difx[:, :, 127::-1]
    imgOOn = difx[:, :, 127::-1] - difx[:, :, 128:256]   # = -imgOO
    imgq = np.concatenate([imgEE, imgEO, imgOE, imgOOn], axis=2)  # [B,128,512]

    # per-core blob [128, BLOB_COLS]
    cx = cosX.reshape(128, NCORES, NT, 128)          # [j, c, t, p]
    sx = sinX.reshape(128, NCORES, NT, 128)
    cy = cosY.reshape(NCORES, NT, 128, 128)          # [c, t, p, w]
    sy = sinY.reshape(NCORES, NT, 128, 128)

    blobs = np.empty((NCORES, 128, BLOB_COLS), dtype=bf)
    blobs[:, :, 0:512] = imgq[0].astype(bf)[None]
    blobs[:, :, 512:1024] = imgq[1].astype(bf)[None]
    # tables: for core c, tile t: cols [cx_t | sx_t | cy_t | sy_t]
    tbl = np.concatenate([
        cx.transpose(1, 2, 0, 3),                    # [c, t, j, p] -> cx block
        sx.transpose(1, 2, 0, 3),
        cy.transpose(0, 1, 2, 3),                    # [c, t, p, w]
        sy.transpose(0, 1, 2, 3),
    ], axis=-1)                                       # [c, t, 128, 512]
    blobs[:, :, TBL0:] = tbl.transpose(0, 2, 1, 3).reshape(
        NCORES, 128, NT * TSTRIDE).astype(bf)

    phase = np.exp(1j * (kx + ky) / 2.0).astype(np.complex64)
    return blobs, phase


def kernel(image, trajectory):
    from concourse.bass_utils import run_bass_kernel_spmd

    if 'nc' not in _CACHE:
        _CACHE['nc'] = _build()
    nc = _CACHE['nc']

    image = np.ascontiguousarray(np.asarray(image, dtype=np.float32))
    trajectory = np.ascontiguousarray(np.asarray(trajectory, dtype=np.float32))
    blobs, phase = _host_prep(image, trajectory)

    in_maps = [{"blob": np.ascontiguousarray(blobs[c])} for c in range(NCORES)]

    res = run_bass_kernel_spmd(nc, in_maps, core_ids=list(range(NCORES)))

    kspace = np.empty((B, M), dtype=np.complex64)
    for c in range(NCORES):
        o = res.results[c]["out"]          # [128, 4*NT]
        o = o.reshape(128, NT, 2, 2)       # [p, t, b, (R, N)]
        for b in range(B):
            R = o[:, :, b, 0].T.reshape(ML)    # m = t*128 + p
            N = o[:, :, b, 1].T.reshape(ML)
            kspace[b, c * ML:(c + 1) * ML] = R - 1j * N
    kspace *= phase[None, :]
    return kspace


# revision 13
# speedup vs baseline: 2.1956x; 1.1058x over previous
"""Type-2 NUFFT (image -> non-uniform k-space) on 8 Trainium2 NeuronCores.

kspace[b,m] = sum_{x,y} image[b,x,y] * exp(-i*(kx_m*(x-128) + ky_m*(y-128)))

Quarter-fold decomposition with half-integer centering: write
x-128 = v - 1/2 with v = x - 127.5 in +-{0.5, ..., 127.5}, and likewise
y-128 = u - 1/2.  Then

  kspace[b,m] = e^{i(kx+ky)/2} * sum_{v,u} img * e^{-i(kx v + ky u)}

and the inner sum folds EXACTLY into 128x128 quadrant images (cos is even,
sin is odd in both v and u):

  inner = R - i*N
  R[m] = sum_w C~E[m,w]*cosY[m,w] - S~O[m,w]*sinY[m,w]
  N[m] = sum_w C~O[m,w]*sinY[m,w] + S~E[m,w]*cosY[m,w]
  C~E = cosX^T @ imgEE   C~O = cosX^T @ imgEO      (per batch)
  S~E = sinX^T @ imgOE   S~O = sinX^T @ imgOO

Work split: the host (numpy) computes the trig tables (cosX/sinX per m-tile,
cosY/sinY per m-tile) and the folded quadrant images in bf16, packs them
into one consumption-ordered blob, and applies the final e^{i(kx+ky)/2}
rotation.  The device does the O(M*N^2) work only: per (batch, m-tile) two
bf16 matmuls into PSUM and two fused DVE multiply+row-reduce ops that
accumulate straight into the output column.
"""

import sys

if '/opt/trn_rl_repo' not in sys.path:
    sys.path.insert(0, '/opt/trn_rl_repo')

import numpy as np
import ml_dtypes

B, NX, NY, M, NCORES = 2, 256, 256, 16384, 8
ML = M // NCORES            # 2048 m-points per core
NT = ML // 128              # 16 m-tiles per core

# blob layout (bf16, per partition-col), ordered by first consumption:
#   [img b0 (512) | t0 tables (512) | img b1 (512) | t1..t15 tables]
# where a t-table block is [cx(128) | sx(128) | w=cosY|sinY (256)].
TSTRIDE = 512
BLOB_COLS = 1536 + (NT - 1) * 512


def _tcol(t):
    """blob column where tile t's table block starts"""
    return 512 if t == 0 else 1536 + (t - 1) * TSTRIDE


def _imgcol(b):
    return 0 if b == 0 else 1024

_CACHE = {}


def _build():
    import concourse.bacc as bacc
    import concourse.bass as bass
    import concourse.mybir as mybir
    from concourse.tile import TileContext

    A = mybir.AluOpType
    f32 = mybir.dt.float32
    bf16 = mybir.dt.bfloat16

    nc = bacc.Bacc("TRN2", target_bir_lowering=False, debug=False)

    blob = nc.dram_tensor("blob", [128, BLOB_COLS], bf16, kind="ExternalInput")
    out = nc.dram_tensor("out", [128, 4 * NT], f32, kind="ExternalOutput")

    def seg2(tile_ap, start, seg_stride):
        """[128, 2, 128] view: two 128-wide segments at start, start+stride."""
        t_ = tile_ap.tensor
        row = tile_ap.ap[0][0]
        return bass.AP(t_, tile_ap.offset + start,
                       [[row, 128], [seg_stride, 2], [1, 128]])

    # DMA chunks over the blob, in consumption order (first small, for a
    # fast pipeline start)
    bounds = [0, 1024, _tcol(2), _tcol(5), _tcol(9), _tcol(13), BLOB_COLS]

    with TileContext(nc) as tc:
        with tc.tile_pool(name="const", bufs=1) as cpool, \
             tc.tile_pool(name="work", bufs=6) as wpool, \
             tc.tile_pool(name="ps", bufs=6, space="PSUM") as ps:

            bsb = cpool.tile([128, BLOB_COLS], bf16, name="blob")
            out_sb = cpool.tile([128, 4 * NT], f32)

            for i in range(len(bounds) - 1):
                cs = slice(bounds[i], bounds[i + 1])
                nc.sync.dma_start(bsb[:, cs], blob[:, cs])

            # per-(t,b) stage-2 path: A = DVE fused multiply+reduce from PSUM;
            # B = Act evicts PSUM->SBUF, Pool multiplies, DVE reduces (2x);
            # C = like B but Act reduces.  Balances DVE/Act/Pool busy time.
            PATTERN = "ACABAACABAACABAACAABACAABACAABAC"
            F = mybir.ActivationFunctionType

            for t in range(NT):
                c0 = _tcol(t)
                for b in range(B):
                    i0 = _imgcol(b)
                    ab = ps.tile([128, 512], f32, tag="ab")
                    # ab = [C~E | C~O | S~E | -S~O]
                    nc.tensor.matmul(ab[:, 0:256],
                                     bsb[:, c0:c0 + 128],
                                     bsb[:, i0:i0 + 256],
                                     start=True, stop=True)
                    nc.tensor.matmul(ab[:, 256:512],
                                     bsb[:, c0 + 128:c0 + 256],
                                     bsb[:, i0 + 256:i0 + 512],
                                     start=True, stop=True)
                    col = t * 4 + b * 2
                    path = PATTERN[t * 2 + b]
                    # R = sum(C~E*cosY) + sum(-S~O*sinY)   -> col
                    # N = sum(C~O*sinY) + sum(S~E*cosY)    -> col+1
                    if path == 'A':
                        scr = wpool.tile([128, 256], f32, tag="scr")
                        scr2 = wpool.tile([128, 256], f32, tag="scr2")
                        nc.vector.scalar_tensor_tensor(
                            seg2(scr[:, :], 0, 128),
                            seg2(ab[:, :], 0, 384), 1.0,
                            seg2(bsb[:, :], c0 + 256, 128),
                            op0=A.mult, op1=A.mult,
                            accum_out=out_sb[:, col:col + 1])
                        nc.vector.scalar_tensor_tensor(
                            seg2(scr2[:, :], 0, 128),
                            seg2(ab[:, :], 128, 128), 1.0,
                            seg2(bsb[:, :], c0 + 384, -128),
                            op0=A.mult, op1=A.mult,
                            accum_out=out_sb[:, col + 1:col + 2])
                    else:
                        cp = wpool.tile([128, 512], f32, tag="cp")
                        nc.scalar.copy(cp[:, :], ab[:, :])
                        p1 = wpool.tile([128, 256], f32, tag="p1")
                        p2 = wpool.tile([128, 256], f32, tag="p2")
                        nc.gpsimd.tensor_tensor(
                            seg2(p1[:, :], 0, 128),
                            seg2(cp[:, :], 0, 384),
                            seg2(bsb[:, :], c0 + 256, 128), op=A.mult)
                        nc.gpsimd.tensor_tensor(
                            seg2(p2[:, :], 0, 128),
                            seg2(cp[:, :], 128, 128),
                            seg2(bsb[:, :], c0 + 384, -128), op=A.mult)
                        if path == 'B':
                            d1 = wpool.tile([128, 256], f32, tag="d1")
                            d2 = wpool.tile([128, 256], f32, tag="d2")
                            nc.vector.tensor_scalar(
                                d1[:, :], p1[:, :], scalar1=1.0, scalar2=0.0,
                                op0=A.mult, op1=A.add,
                                accum_out=out_sb[:, col:col + 1])
                            nc.vector.tensor_scalar(
                                d2[:, :], p2[:, :], scalar1=1.0, scalar2=0.0,
                                op0=A.mult, op1=A.add,
                                accum_out=out_sb[:, col + 1:col + 2])
                        else:
                            d1 = wpool.tile([128, 256], f32, tag="d1")
                            d2 = wpool.tile([128, 256], f32, tag="d2")
                            nc.scalar.activation(
                                d1[:, :], p1[:, :], F.Copy,
                                accum_out=out_sb[:, col:col + 1])
                            nc.scalar.activation(
                                d2[:, :], p2[:, :], F.Copy,
                                accum_out=out_sb[:, col + 1:col + 2])
                if t % 4 == 3:
                    qs = slice((t - 3) * 4, (t + 1) * 4)
                    nc.sync.dma_start(out[:, qs], out_sb[:, qs])

    nc.compile()
    return nc


def _host_prep(image, trajectory):
    """Folded quadrant images + trig tables (bf16) packed per-core blobs."""
    bf = ml_dtypes.bfloat16
    kx = trajectory[0].astype(np.float32)            # [M]
    ky = trajectory[1].astype(np.float32)
    v = (np.arange(128, dtype=np.float32) + 0.5)

    cosX = np.cos(kx[None, :] * v[:, None])          # [128, M]
    sinX = np.sin(kx[None, :] * v[:, None])
    argY = ky[:, None] * v[None, :]                  # [M, 128]
    cosY = np.cos(argY)
    sinY = np.sin(argY)

    # quadrant folds (x: rows about 127.5; y: cols about 127.5)
    top = image[:, 128:256, :]
    bot = image[:, 127::-1, :]
    sumx = top + bot
    difx = top - bot
    imgEE = sumx[:, :, 128:256] + sumx[:, :, 127::-1]
    imgEO = sumx[:, :, 128:256] - sumx[:, :, 127::-1]
    imgOE = difx[:, :, 128:256] + difx[:, :, 127::-1]
    imgOOn = difx[:, :, 127::-1] - difx[:, :, 128:256]   # = -imgOO
    imgq = np.concatenate([imgEE, imgEO, imgOE, imgOOn], axis=2)  # [B,128,512]

    # per-core blob [128, BLOB_COLS]
    cx = cosX.reshape(128, NCORES, NT, 128)          # [j, c, t, p]
    sx = sinX.reshape(128, NCORES, NT, 128)
    cy = cosY.reshape(NCORES, NT, 128, 128)          # [c, t, p, w]
    sy = sinY.reshape(NCORES, NT, 128, 128)

    blobs = np.empty((NCORES, 128, BLOB_COLS), dtype=bf)
    blobs[:, :, 0:512] = imgq[0].astype(bf)[None]
    blobs[:, :, 1024:1536] = imgq[1].astype(bf)[None]
    # tables: for core c, tile t: cols [cx_t | sx_t | cy_t | sy_t]
    tbl = np.concatenate([
        cx.transpose(1, 2, 0, 3),                    # [c, t, j, p] -> cx block
        sx.transpose(1, 2, 0, 3),
        cy.transpose(0, 1, 2, 3),                    # [c, t, p, w]
        sy.transpose(0, 1, 2, 3),
    ], axis=-1).astype(bf)                            # [c, t, 128, 512]
    tblp = tbl.transpose(0, 2, 1, 3)                  # [c, 128, t, 512]
    blobs[:, :, 512:1024] = tblp[:, :, 0, :]
    blobs[:, :, 1536:] = tblp[:, :, 1:, :].reshape(NCORES, 128,
                                                   (NT - 1) * TSTRIDE)

    phase = np.exp(1j * (kx + ky) / 2.0).astype(np.complex64)
    return blobs, phase


def kernel(image, trajectory):
    from concourse.bass_utils import run_bass_kernel_spmd

    if 'nc' not in _CACHE:
        _CACHE['nc'] = _build()
    nc = _CACHE['nc']

    image = np.ascontiguousarray(np.asarray(image, dtype=np.float32))
    trajectory = np.ascontiguousarray(np.asarray(trajectory, dtype=np.float32))
    blobs, phase = _host_prep(image, trajectory)

    in_maps = [{"blob": np.ascontiguousarray(blobs[c])} for c in range(NCORES)]

    res = run_bass_kernel_spmd(nc, in_maps, core_ids=list(range(NCORES)))

    kspace = np.empty((B, M), dtype=np.complex64)
    for c in range(NCORES):
        o = res.results[c]["out"]          # [128, 4*NT]
        o = o.reshape(128, NT, 2, 2)       # [p, t, b, (R, N)]
        for b in range(B):
            R = o[:, :, b, 0].T.reshape(ML)    # m = t*128 + p
            N = o[:, :, b, 1].T.reshape(ML)
            kspace[b, c * ML:(c + 1) * ML] = R - 1j * N
    kspace *= phase[None, :]
    return kspace
